# revision 86
# baseline (speedup 1.0000x reference)
"""Trainium2 Bass kernel for the caption-generation module (2-layer GRU
encoder-decoder + vocab projection + log_softmax).

Strategy: data-parallel over batch across 8 NeuronCores (B=128 -> 16 rows
per core, weights replicated).  Per core, everything runs in a transposed
layout (feature dim on SBUF partitions, (time*batch) on the free dim):

  E1:  gi1[t] = x_t @ w_ih1.T for all 40 encoder steps  (one batched matmul)
  C1:  h1 chain, 67 sequential steps, only h1 @ w_hh1.T inside the loop
       (decoder rnn1 input is zero so its gi is just the bias)
  E3:  gi2[t] = [h1_t; w_t] @ w_ih2.T for all 67 steps  (batched matmul)
  C2:  h2 chain, 67 sequential steps
  P :  logits = h2_dec @ out_w.T + out_b, then streamed log_softmax,
       DMA straight to the output

Matmul inputs are cast to bf16 (fp32 accumulate in PSUM); gate math and
softmax run in fp32.
"""

import sys
import types

sys.path.insert(0, "/opt/trn_rl_repo")

import numpy as np
import ml_dtypes

import concourse.bass as bass
import concourse.mybir as mybir
import concourse.tile as tile
from concourse.alu_op_type import AluOpType
from concourse.vector_clock import ScopedClock

BF16 = mybir.dt.bfloat16
F32 = mybir.dt.float32
F8 = mybir.dt.float8e3
F8E4 = mybir.dt.float8e4
WSCALE = 128.0  # fp8 chain-weight pre-scale (host multiplies, gates divide)
HSCALE = 8.0    # fp8 h shadow pre-scale (|h| <= 1 -> 8 < e3m4 max 15.5)
PSCALE = 64.0   # fp8e4 h2 pre-scale for the vocab projection
OWSCALE = 64.0  # fp8e4 out_w pre-scale (host multiplies)
AF = mybir.ActivationFunctionType
PROJ_IN_CHAIN = 0  # mtiles spread into the chain tail


# ---------------------------------------------------------------------------
# Workaround: this container's walrus rejects CTRL instructions carrying more
# than one sync-wait command.  Split the TileContext tail drain's wait list
# across a chain of drains, one wait each.
# ---------------------------------------------------------------------------
def _patched_drain_and_barrier(self, tick_clock, wait_clock):
    import bass_rust

    drain_inst = self.nc.sync.drain()
    wait_clock.add_sem_waits(
        drain_inst.ins, ScopedClock({None: tick_clock.global_clock})
    )
    waits = list(drain_inst.ins.sync_info.on_wait)
    if len(waits) > 1:
        si = drain_inst.ins.sync_info
        si.on_wait = waits[:1]
        drain_inst.ins.sync_info = si
        for i in range(1, len(waits)):
            extra = self.nc.sync.drain()
            extra.ins.sync_info = bass_rust.SyncInfo(
                on_wait=waits[i : i + 1], on_update=[]
            )
    self.nc.all_engine_barrier()
    assert self.sems is not None
    popped = self.nc._tile_sem_poison_stack.pop()
    assert popped is self._sem_poison
    self.nc.clear_and_free_semaphores(list(self.sems.allocated().values()))
    self.nc.all_engine_barrier()


tile.TileContext._drain_and_barrier = _patched_drain_and_barrier

# Same walrus limitation for regular engine instructions: at most one
# sync-wait per instruction.  Split extra waits onto preceding NoOps on the
# same engine (engine stalls there instead — identical semantics).
_orig_commit = tile.TileContext._commit_instruction


def _commit_split_waits(self, inst, lazy_reg_writes=True):
    si = getattr(inst, "sync_info", None)
    if (si is not None and si.on_wait and len(si.on_wait) > 1
            and inst.engine != mybir.EngineType.Unassigned):
        waits = list(si.on_wait)
        for w in waits[:-1]:
            nop = mybir.InstNoOp(
                name=self.nc.get_next_instruction_name(),
                sync_info=mybir.SyncInfo(on_wait=[w], on_update=[]),
                bass_nofuse=True,
                engine=inst.engine,
            )
            _orig_commit(self, nop, lazy_reg_writes=False)
        si.on_wait = waits[-1:]
        inst.sync_info = si
    return _orig_commit(self, inst, lazy_reg_writes)


tile.TileContext._commit_instruction = _commit_split_waits


# ---------------------------------------------------------------------------
# Config
# ---------------------------------------------------------------------------
def make_cfg(B=128, NF=40, TD=27, V=16000, DV=2048, DH=512, DW=512,
             n_cores=8, has_out_b=False, chain_mode="fp32", has_bhn=False):
    cfg = dict(B=B, NF=NF, TD=TD, V=V, DV=DV, DH=DH, DW=DW,
               n_cores=n_cores, has_out_b=has_out_b, chain_mode=chain_mode,
               has_bhn=has_bhn)
    cfg["BS"] = B // n_cores          # batch rows per core
    cfg["KV"] = DV // 128             # x feature chunks
    cfg["KH"] = DH // 128             # h feature chunks
    cfg["KW"] = DW // 128             # word feature chunks
    cfg["MC"] = 3 * DH // 128         # gate chunks
    cfg["NSTEP"] = NF + TD            # total chain steps
    cfg["ROWS_E"] = NF * cfg["BS"]    # encoder (t,b) columns
    cfg["ROWS_A"] = cfg["NSTEP"] * cfg["BS"]
    cfg["ROWS_D"] = TD * cfg["BS"]    # decode (t,b) columns
    # vocab tiling for the projection (psum free dim <= 512 fp32)
    for pn in (512, 500, 400, 320, 256):
        if V % pn == 0:
            cfg["PN"] = pn
            break
    else:
        raise ValueError(f"V={V} has no tile size")
    cfg["VCH"] = V // 4               # log_softmax streaming chunk
    return cfg


def _ntiles(total, maxn):
    """Split `total` into tiles of at most maxn (last ragged)."""
    out = []
    n0 = 0
    while n0 < total:
        nn = min(maxn, total - n0)
        out.append((n0, nn))
        n0 += nn
    return out


# ---------------------------------------------------------------------------
# Kernel builder
# ---------------------------------------------------------------------------
def build_nc(cfg):
    BS, KV, KH, KW, MC = cfg["BS"], cfg["KV"], cfg["KH"], cfg["KW"], cfg["MC"]
    NF, TD, V, DH = cfg["NF"], cfg["TD"], cfg["V"], cfg["DH"]
    NSTEP, ROWS_E, ROWS_D = cfg["NSTEP"], cfg["ROWS_E"], cfg["ROWS_D"]
    PN, VCH = cfg["PN"], cfg["VCH"]
    G3 = 3 * DH
    LAG = 12  # h2 trails h1: > E3 block size + slack for spread-out pieces
    FAST = cfg["chain_mode"] == "fp8" and not cfg["has_bhn"]
    XSCALE = 16.0   # fp8e4 video-feature pre-scale for E1
    W1SCALE = 64.0  # fp8e4 w_ih1 pre-scale for E1

    nc = bass.Bass()

    # ---- DRAM parameters (per-core views; host prepares these) ----
    # xT/w1T are packed partition-major ([128, k*n]) so the boot DMA moves
    # ~10-25KB contiguous runs per partition instead of 640B rows
    xT = nc.dram_tensor("xT", [128, KV * ROWS_E], F8E4, kind="ExternalInput")
    wordsT = nc.dram_tensor("wordsT", [cfg["DW"], ROWS_D], BF16, kind="ExternalInput")
    w1T = nc.dram_tensor("w1T", [128, KV * G3], F8E4, kind="ExternalInput")
    chain_wdt = {"fp8": F8, "bf16": BF16, "fp32": F32}[cfg["chain_mode"]]
    chain_hdt = F32 if cfg["chain_mode"] == "fp32" else BF16
    wh1T = nc.dram_tensor("wh1T", [DH, G3], chain_wdt, kind="ExternalInput")
    w2T = nc.dram_tensor("w2T", [DH + cfg["DW"], G3], BF16, kind="ExternalInput")
    wh2T = nc.dram_tensor("wh2T", [DH, G3], chain_wdt, kind="ExternalInput")
    owf = nc.dram_tensor("owf", [DH, V], F8E4, kind="ExternalInput")
    bi1c = nc.dram_tensor("bi1c", [128, MC], F32, kind="ExternalInput")
    bi2c = nc.dram_tensor("bi2c", [128, MC], F32, kind="ExternalInput")
    gidec = nc.dram_tensor("gidec", [128, MC, BS],
                           F8E4 if FAST else F32, kind="ExternalInput")
    bhn1 = nc.dram_tensor("bhn1", [128, KH, BS], F32, kind="ExternalInput")
    bhn2 = nc.dram_tensor("bhn2", [128, KH, BS], F32, kind="ExternalInput")
    if cfg["has_out_b"]:
        outb = nc.dram_tensor("outb", [1, V], BF16, kind="ExternalInput")
        ones = nc.dram_tensor("ones", [1, 128], BF16, kind="ExternalInput")
    out = nc.dram_tensor("out", [BS, TD, V], BF16, kind="ExternalOutput")
    # view [t, b, v] of out[b, t, v] (strides V, TD*V, 1); row r = t*BS + b
    _o = out[:]
    out_tbv = bass.AP(tensor=_o.tensor, offset=_o.offset,
                      ap=[[V, TD], [TD * V, BS], [1, V]])

    def out_slice(r0, mrows, c0, cw):
        assert r0 % BS == 0 and mrows % BS == 0
        return out_tbv[r0 // BS:(r0 + mrows) // BS, :, c0:c0 + cw]

    from contextlib import ExitStack

    with tile.TileContext(nc) as tc:
        with (
            tc.tile_pool(name="pconst", bufs=1) as pconst,
            tc.tile_pool(name="pchain", bufs=6) as pchain,
            tc.tile_pool(name="psum_mm", bufs=3, space="PSUM") as psum_mm,
            tc.tile_pool(name="psum_gh", bufs=2, space="PSUM") as psum_gh,
            tc.tile_pool(name="ph2", bufs=1) as ph2,
            tc.tile_pool(name="powf", bufs=1) as powf,
            tc.tile_pool(name="pp", bufs=1) as pp,
            tc.tile_pool(name="plg", bufs=2) as plg,
            tc.tile_pool(name="pedump", bufs=3) as pedump,
            tc.tile_pool(name="psmall", bufs=3) as psmall,
        ):
            # ---- constants ----
            bi1c_sb = pconst.tile([128, MC], F32, tag="bi1c")
            nc.sync.dma_start(out=bi1c_sb[:], in_=bi1c[:])
            bi2c_sb = pconst.tile([128, MC], F32, tag="bi2c")
            nc.sync.dma_start(out=bi2c_sb[:], in_=bi2c[:])
            gidec_sb = pconst.tile([128, MC, BS], F8E4 if FAST else F32,
                                   tag="gidec")
            nc.sync.dma_start(out=gidec_sb[:], in_=gidec[:])
            bhn1_sb = pconst.tile([128, KH, BS], F32, tag="bhn1")
            nc.sync.dma_start(out=bhn1_sb[:], in_=bhn1[:])
            bhn2_sb = pconst.tile([128, KH, BS], F32, tag="bhn2")
            nc.sync.dma_start(out=bhn2_sb[:], in_=bhn2[:])

            if cfg["has_out_b"]:
                outb_sb = pconst.tile([1, V], BF16, tag="outb")
                nc.sync.dma_start(out=outb_sb[:], in_=outb[:])
                ones_sb = pconst.tile([1, 128], BF16, tag="ones")
                nc.sync.dma_start(out=ones_sb[:], in_=ones[:])

            if FAST:
                # h2 state lives only in fp8 (xHSCALE); no bf16 copy needed
                h2_sb = h2b_sb = None
                h2f_sb = ph2.tile([128, KH, (NSTEP + 1) * BS], F8, tag="h2f")
                nc.vector.memset(h2f_sb[:, :, 0:BS], 0.0)
            elif cfg["chain_mode"] == "fp8":
                h2_sb = ph2.tile([128, KH, (NSTEP + 1) * BS], chain_hdt,
                                 tag="h2")
                nc.vector.memset(h2_sb[:, :, 0:BS], 0.0)
                h2b_sb = h2_sb
                h2f_sb = ph2.tile([128, KH, (NSTEP + 1) * BS], F8, tag="h2f")
                nc.vector.memset(h2f_sb[:, :, 0:BS], 0.0)
            else:
                h2_sb = ph2.tile([128, KH, (NSTEP + 1) * BS], chain_hdt,
                                 tag="h2")
                nc.vector.memset(h2_sb[:, :, 0:BS], 0.0)
                if cfg["chain_mode"] == "fp32":
                    h2b_sb = ph2.tile([128, KH, (NSTEP + 1) * BS], BF16,
                                      tag="h2b")
                    nc.vector.memset(h2b_sb[:, :, 0:BS], 0.0)
                else:
                    h2b_sb = h2_sb
                h2f_sb = None
            # ---------------- gate math shared by both chains -------------
            # Both chains' steps are emitted STAGE-STAGGERED: each engine's
            # FIFO is strict in-order, so chain A's stalled op at the queue
            # head would otherwise block chain B's ready ops behind it and
            # the two serial gate chains would run back-to-back instead of
            # overlapped.  fp8 fast path (bhh == 0): h state is carried as
            # fp8e3 scaled by HSCALE, written directly by the last DVE op;
            # h1 additionally gets a bf16 (xHSCALE) copy for E3.
            fast_gate = FAST

            def gate_pair(steps):
                ws = (1.0 / (WSCALE * HSCALE)) if cfg["chain_mode"] == "fp8" else 1.0
                n = len(steps)
                box = [dict() for _ in range(n)]

                def tl(i, tag, ch, dt=BF16):
                    # bf16 intermediates: 2x DVE rate on the all-16-bit ops
                    return pchain.tile([128, ch, BS], dt, tag=tag,
                                       name=f"{tag}{i}")

                def s_rz(i):
                    st = steps[i]
                    b = box[i]
                    b["rz"] = tl(i, "rz", 2 * KH, F32)
                    nc.vector.scalar_tensor_tensor(
                        out=b["rz"][:], in0=st["ghrz"][:], scalar=ws,
                        in1=st["gi"][:, 0:2 * KH, :],
                        op0=AluOpType.mult, op1=AluOpType.add)

                def s_sig(i):
                    b = box[i]
                    b["rzs"] = tl(i, "rzs", 2 * KH)
                    nc.scalar.activation(out=b["rzs"][:], in_=b["rz"][:],
                                         func=AF.Sigmoid)

                def s_zc(i):
                    # zc = 1 - z on DVE, off the critical path
                    b = box[i]
                    if not fast_gate:
                        return
                    b["zc"] = tl(i, "zc", KH)
                    nc.vector.tensor_scalar(
                        out=b["zc"][:], in0=b["rzs"][:, KH:, :],
                        scalar1=-1.0, scalar2=1.0,
                        op0=AluOpType.mult, op1=AluOpType.add)

                def s_np0(i):
                    st = steps[i]
                    b = box[i]
                    b["np0"] = tl(i, "np0", KH)
                    if fast_gate:
                        nc.vector.scalar_tensor_tensor(
                            out=b["np0"][:], in0=st["ghn"][:], scalar=ws,
                            in1=b["rzs"][:, 0:KH, :],
                            op0=AluOpType.mult, op1=AluOpType.mult)
                    else:
                        tmpn = tl(i, "tmpn", KH)
                        nc.vector.scalar_tensor_tensor(
                            out=tmpn[:], in0=st["ghn"][:], scalar=ws,
                            in1=st["bhh"][:],
                            op0=AluOpType.mult, op1=AluOpType.add)
                        nc.vector.tensor_tensor(
                            out=b["np0"][:], in0=b["rzs"][:, 0:KH, :],
                            in1=tmpn[:], op=AluOpType.mult)

                def s_np1(i):
                    st = steps[i]
                    b = box[i]
                    b["np1"] = tl(i, "np1", KH)
                    nc.vector.tensor_tensor(
                        out=b["np1"][:], in0=st["gi"][:, 2 * KH:, :],
                        in1=b["np0"][:], op=AluOpType.add)

                def s_tanh(i):
                    b = box[i]
                    b["nt"] = tl(i, "nt", KH)
                    nc.scalar.activation(out=b["nt"][:], in_=b["np1"][:],
                                         func=AF.Tanh)

                def s_hm0(i):
                    st = steps[i]
                    b = box[i]
                    b["hm0"] = tl(i, "hm0", KH)
                    if fast_gate:
                        # t1 = z * prev8: needs only sig + old state, so it
                        # runs parallel to the np0/np1/tanh path
                        nc.vector.tensor_tensor(
                            out=b["hm0"][:], in0=b["rzs"][:, KH:, :],
                            in1=st["prev"], op=AluOpType.mult)
                    else:
                        nc.vector.tensor_tensor(
                            out=b["hm0"][:], in0=st["prev"], in1=b["nt"][:],
                            op=AluOpType.subtract)

                def s_hm1(i):
                    b = box[i]
                    b["hm1"] = tl(i, "hm1", KH)
                    if fast_gate:
                        # t2 = (1-z) * nt*HS
                        nc.vector.scalar_tensor_tensor(
                            out=b["hm1"][:], in0=b["nt"][:], scalar=HSCALE,
                            in1=b["zc"][:], op0=AluOpType.mult,
                            op1=AluOpType.mult)
                    else:
                        nc.vector.tensor_tensor(
                            out=b["hm1"][:], in0=b["rzs"][:, KH:, :],
                            in1=b["hm0"][:], op=AluOpType.mult)

                def s_h(i):
                    st = steps[i]
                    b = box[i]
                    t = st["t"]
                    if fast_gate:
                        # h8 = t1 + t2 -> f8 state t+1
                        nc.vector.tensor_tensor(
                            out=st["f8"][:, :, (t + 1) * BS:(t + 2) * BS],
                            in0=b["hm0"][:], in1=b["hm1"][:],
                            op=AluOpType.add)
                    else:
                        nc.vector.tensor_tensor(
                            out=st["save"][:, :, (t + 1) * BS:(t + 2) * BS],
                            in0=b["nt"][:], in1=b["hm1"][:], op=AluOpType.add)

                def s_post(i):
                    st = steps[i]
                    t = st["t"]
                    if fast_gate:
                        if st["bf"] is not None:
                            nc.scalar.copy(
                                out=st["bf"][:, :, (t + 1) * BS:(t + 2) * BS],
                                in_=st["f8"][:, :, (t + 1) * BS:(t + 2) * BS])
                    else:
                        if st["f8"] is not None:
                            nc.scalar.mul(
                                out=st["f8"][:, :, (t + 1) * BS:(t + 2) * BS],
                                in_=st["save"][:, :, (t + 1) * BS:(t + 2) * BS],
                                mul=HSCALE)
                        if st["shadow"] is not None:
                            nc.vector.tensor_copy(
                                out=st["shadow"][:, :, (t + 1) * BS:(t + 2) * BS],
                                in_=st["save"][:, :, (t + 1) * BS:(t + 2) * BS])

                stages = [s_rz, s_sig, s_zc, s_np0, s_np1, s_tanh,
                          s_hm0, s_hm1, s_h, s_post]
                if fast_gate:
                    # zc and t1 (hm0) fill DVE while ACT runs tanh
                    seq = [0, 1, 3, 4, 2, 6, 5, 7, 8, 9]
                else:
                    seq = [0, 1, 3, 4, 5, 6, 7, 8, 9]
                if n == 2:
                    order = [(0, s) for s in seq[:5]]
                    tail1 = seq[5:]
                    for j, s in enumerate(seq):
                        order.append((1, s))
                        if j < len(tail1):
                            order.append((0, tail1[j]))
                else:
                    order = [(0, s) for s in seq]
                for i, s in order:
                    stages[s](i)

            def recur_matmul(whh_sb, rhs_sb, t, gi):
                # r/z gates in their own psum tile so the first gate op can
                # start after their matmuls
                ghrz = psum_gh.tile([128, 2 * KH, BS], F32, tag="ghrz")
                ghn = psum_gh.tile([128, KH, BS], F32, tag="ghn")
                prev = rhs_sb[:, :, t * BS:(t + 1) * BS]
                for m in range(MC):
                    dst = (ghrz[:, m, :] if m < 2 * KH
                           else ghn[:, m - 2 * KH, :])
                    for k in range(KH):
                        nc.tensor.matmul(
                            dst,
                            lhsT=whh_sb[:, k, m * 128:(m + 1) * 128],
                            rhs=prev[:, k, :],
                            start=(k == 0), stop=(k == KH - 1))
                return ghrz, ghn

            # ====== E1, interleaved h1/E3/h2 chains, projection ======
            with ExitStack() as chain_es:
                pmidA = chain_es.enter_context(tc.tile_pool(name="pmidA", bufs=1))
                h1_sb = pmidA.tile([128, KH, (NSTEP + 1) * BS], chain_hdt, tag="h1")
                nc.vector.memset(h1_sb[:, :, 0:BS], 0.0)
                if cfg["chain_mode"] == "fp32":
                    h1b_sb = pmidA.tile([128, KH, (NSTEP + 1) * BS], BF16,
                                        tag="h1b")
                    nc.vector.memset(h1b_sb[:, :, 0:BS], 0.0)
                else:
                    h1b_sb = h1_sb
                if cfg["chain_mode"] == "fp8":
                    h1f_sb = pmidA.tile([128, KH, (NSTEP + 1) * BS], F8,
                                        tag="h1f")
                    nc.vector.memset(h1f_sb[:, :, 0:BS], 0.0)
                else:
                    h1f_sb = None
                gi1_sb = pmidA.tile([128, MC, ROWS_E],
                                    F8E4 if FAST else BF16, tag="gi1")
                wh1_sb = pmidA.tile([128, KH, G3], chain_wdt, tag="wh1")

                with tc.tile_pool(name="pw1", bufs=1) as pw1:
                    x_sb = pw1.tile([128, KV, ROWS_E], F8E4, tag="x")
                    xT_r = xT[:].rearrange("p (k n) -> p k n", k=KV)
                    for g in range(8):
                        sl = slice(16 * g, 16 * (g + 1))
                        nc.sync.dma_start(out=x_sb[sl], in_=xT_r[sl])
                    w1_sb = pw1.tile([128, KV, G3], F8E4, tag="w1")
                    w1T_r = w1T[:].rearrange("p (k n) -> p k n", k=KV)
                    for g in range(8):
                        sl = slice(16 * g, 16 * (g + 1))
                        nc.sync.dma_start(out=w1_sb[sl], in_=w1T_r[sl])
                    wh1T_r = wh1T[:].rearrange("(k p) n -> p k n", p=128)
                    for k in range(KH):
                        nc.sync.dma_start(out=wh1_sb[:, k, :], in_=wh1T_r[:, k, :])

                    # E1: gi1 = w1T.T @ x, fp8e4 DoubleRow (2 k-chunks per
                    # matmul); the bias ACT divides out the fp8 pre-scales
                    for (n0, nn) in _ntiles(ROWS_E, 320):
                        for m in range(MC):
                            ps = psum_mm.tile([128, 512], F32, tag="mm")
                            for c in range(KV // 2):
                                nc.tensor.matmul(
                                    ps[:, :nn],
                                    lhsT=w1_sb[:, 2 * c:2 * c + 2,
                                               m * 128:(m + 1) * 128],
                                    rhs=x_sb[:, 2 * c:2 * c + 2, n0:n0 + nn],
                                    start=(c == 0), stop=(c == KV // 2 - 1),
                                    perf_mode=mybir.MatmulPerfMode.DoubleRow)
                            nc.scalar.activation(
                                out=gi1_sb[:, m, n0:n0 + nn], in_=ps[:, :nn],
                                func=AF.Identity, bias=bi1c_sb[:, m:m + 1],
                                scale=1.0 / (XSCALE * W1SCALE))

                # layer-2 weights / words / gi2 (loaded while chains run)
                pmidB = chain_es.enter_context(tc.tile_pool(name="pmidB", bufs=1))
                w2_sb = pmidB.tile([128, KH + KW, G3], BF16, tag="w2")
                w2T_r = w2T[:].rearrange("(k p) n -> p k n", p=128)
                for k in range(KH + KW):
                    nc.sync.dma_start(out=w2_sb[:, k, :], in_=w2T_r[:, k, :])
                words_sb = pmidB.tile([128, KW, ROWS_D], BF16, tag="words")
                wordsT_r = wordsT[:].rearrange("(k p) n -> p k n", p=128)
                for k in range(KW):
                    nc.sync.dma_start(out=words_sb[:, k, :], in_=wordsT_r[:, k, :])
                wh2_sb = pmidB.tile([128, KH, G3], chain_wdt, tag="wh2")
                wh2T_r = wh2T[:].rearrange("(k p) n -> p k n", p=128)
                for k in range(KH):
                    nc.sync.dma_start(out=wh2_sb[:, k, :], in_=wh2T_r[:, k, :])
                gi2_sb = pmidB.tile([128, MC, NSTEP * BS],
                                    F8E4 if FAST else BF16, tag="gi2")
                # vocab projection weights stream in behind the chain
                owf_sb = powf.tile([128, KH, V], F8E4, tag="owf")
                owf_r = owf[:].rearrange("(k p) n -> p k n", p=128)
                for k in range(KH):
                    nc.sync.dma_start(out=owf_sb[:, k, :], in_=owf_r[:, k, :])

                # ---- projection pieces (mtile 0 is spread into the chain
                # tail; the rest runs after the chain loop) ----
                nvt = V // PN
                dcol0 = (NF + 1) * BS  # first decode h2 col
                mtiles = _ntiles(ROWS_D, 128)
                ISS = 1.0 / (PSCALE * OWSCALE)
                # extra logits downscale so f8e4 storage can't overflow
                # (|logit| up to ~3.7 maps to <= 240)
                LSC = 64.0
                h2p = pp.tile([128, KH, ROWS_D], F8E4, tag="h2p")
                logits_t = {}
                sums_t = {}

                def proj_h2p_piece(mt):
                    r0, mrows = mtiles[mt]
                    if h2f_sb is not None:  # fp8: h2 state is f8 xHSCALE
                        nc.scalar.mul(
                            out=h2p[:, :, r0:r0 + mrows],
                            in_=h2f_sb[:, :, dcol0 + r0:dcol0 + r0 + mrows],
                            mul=PSCALE / HSCALE)
                    else:
                        nc.scalar.mul(
                            out=h2p[:, :, r0:r0 + mrows],
                            in_=h2b_sb[:, :, dcol0 + r0:dcol0 + r0 + mrows],
                            mul=PSCALE)
                    logits_t[mt] = plg.tile([128, V], F8E4, tag="logits",
                                            name=f"logits{mt}")
                    sums_t[mt] = psmall.tile([128, nvt], F32, tag="sums",
                                             name=f"sums{mt}")

                def proj_vtile_piece(mt, nt_i):
                    r0, mrows = mtiles[mt]
                    n0 = nt_i * PN
                    ps = psum_mm.tile([128, 512], F32, tag="mm")
                    last = KH - 1 if not cfg["has_out_b"] else None
                    for k in range(KH):
                        nc.tensor.matmul(
                            ps[:mrows, :PN],
                            lhsT=h2p[:, k, r0:r0 + mrows],
                            rhs=owf_sb[:, k, n0:n0 + PN],
                            start=(k == 0), stop=(k == last))
                    if cfg["has_out_b"]:
                        nc.tensor.matmul(
                            ps[:mrows, :PN],
                            lhsT=ones_sb[:, :mrows],
                            rhs=outb_sb[:, n0:n0 + PN],
                            start=False, stop=True)
                    edump = pedump.tile([128, PN], BF16, tag="edump")
                    nc.scalar.activation(
                        out=edump[:mrows, :], in_=ps[:mrows, :PN],
                        func=AF.Exp, scale=ISS,
                        accum_out=sums_t[mt][:mrows, nt_i:nt_i + 1])
                    nc.vector.tensor_scalar_mul(
                        logits_t[mt][:mrows, n0:n0 + PN], ps[:mrows, :PN],
                        1.0 / LSC)

                def proj_finish(mt, pstage):
                    """lse, then logp = logits*(LSC*ISS) - lse, bf16 out."""
                    r0, mrows = mtiles[mt]
                    NCH = 4  # 4000-col chunks -> 8KB contiguous DMA packets
                    CW = V // NCH
                    s1 = psmall.tile([128, 1], F32, tag="s1")
                    nc.vector.tensor_reduce(
                        out=s1[:mrows], in_=sums_t[mt][:mrows, :],
                        axis=mybir.AxisListType.X, op=AluOpType.add)
                    nshift = psmall.tile([128, 1], F32, tag="nshift")
                    nc.scalar.activation(
                        out=nshift[:mrows], in_=s1[:mrows], func=AF.Ln)
                    nc.vector.tensor_scalar_mul(
                        nshift[:mrows], nshift[:mrows], -1.0)
                    for c in range(NCH):
                        # DVE runs these ~3x faster than ACT (2x 8/16-bit
                        # rate), and ACT is already exp-bound in the MM phase
                        stage = pstage.tile([128, CW], BF16, tag="stage")
                        src = logits_t[mt][:mrows, c * CW:(c + 1) * CW]
                        nc.vector.tensor_scalar(
                            out=stage[:mrows, :], in0=src,
                            scalar1=LSC * ISS, scalar2=nshift[:mrows],
                            op0=AluOpType.mult, op1=AluOpType.add)
                        nc.gpsimd.dma_start(
                            out=out_slice(r0, mrows, c * CW, CW),
                            in_=stage[:mrows, :])

                def h1_mm(t):
                    gi = (gi1_sb[:, :, t * BS:(t + 1) * BS] if t < NF
                          else gidec_sb[:])
                    ghrz, ghn = recur_matmul(
                        wh1_sb, h1f_sb if h1f_sb is not None else h1_sb, t,
                        gi)
                    # FAST: h1_sb carries the bf16 xHSCALE copy (for E3);
                    # prev (f8/bf16 xHSCALE state) feeds t1 = z*prev8
                    psrc = h1f_sb if FAST else h1_sb
                    prev = psrc[:, :, t * BS:(t + 1) * BS]
                    return dict(t=t, ghrz=ghrz, ghn=ghn, gi=gi, bhh=bhn1_sb,
                                prev=prev, save=h1_sb, f8=h1f_sb, bf=h1_sb,
                                shadow=(h1b_sb if h1b_sb is not h1_sb
                                        else None))

                def h2_mm(t):
                    gi = gi2_sb[:, :, t * BS:(t + 1) * BS]
                    ghrz, ghn = recur_matmul(
                        wh2_sb, h2f_sb if h2f_sb is not None else h2_sb, t,
                        gi)
                    psrc = h2f_sb if FAST else h2_sb
                    return dict(t=t, ghrz=ghrz, ghn=ghn, gi=gi,
                                bhh=bhn2_sb,
                                prev=psrc[:, :, t * BS:(t + 1) * BS],
                                save=h2_sb, f8=h2f_sb, bf=None,
                                shadow=(h2b_sb if h2_sb is not None and
                                        h2b_sb is not h2_sb else None))

                # fp8: h1_sb and (host-prescaled) words carry xHSCALE, so the
                # E3 psum is xHSCALE; the bias ACT divides it back out for
                # the n chunks and keeps the xHSCALE(=xQG) for r/z chunks
                # (stored pre-scaled for the identity-matmul accumulate).
                e3_scale = (1.0 / HSCALE) if fast_gate else 1.0

                def e3_piece(t0, nsteps, m):
                    """gi2 chunk m for chain steps [t0, t0+nsteps)."""
                    n0 = t0 * BS
                    nn = nsteps * BS
                    enc = t0 < NF  # blocks never straddle NF
                    ps = psum_mm.tile([128, 512], F32, tag="mm")
                    for k in range(KH):
                        nc.tensor.matmul(
                            ps[:, :nn],
                            lhsT=w2_sb[:, k, m * 128:(m + 1) * 128],
                            rhs=h1b_sb[:, k, BS + n0:BS + n0 + nn],
                            start=(k == 0),
                            stop=(enc and k == KH - 1))
                    if not enc:
                        w0 = n0 - ROWS_E
                        for k in range(KW):
                            nc.tensor.matmul(
                                ps[:, :nn],
                                lhsT=w2_sb[:, KH + k, m * 128:(m + 1) * 128],
                                rhs=words_sb[:, k, w0:w0 + nn],
                                start=False, stop=(k == KW - 1))
                    nc.scalar.activation(
                        out=gi2_sb[:, m, n0:n0 + nn], in_=ps[:, :nn],
                        func=AF.Identity, bias=bi2c_sb[:, m:m + 1],
                        scale=e3_scale)

                # per tt: both chains' matmul bursts, then both gate chains
                # stage-staggered.  E3 work is spread out ~2 one-m-chunk
                # pieces per tt (a full block's 12 ACT ops at once would
                # stall the gate sigmoids in the ACT FIFO); pieces for block
                # [t0, t0+nn) become legal at tt=t0+nn and must finish
                # before tt=t0+LAG.
                blocks = ([(t0, nn) for (t0, nn) in _ntiles(NF, 6)] +
                          [(NF + t0, nn) for (t0, nn) in _ntiles(TD, 6)])
                block_end = {t0 + nn: (t0, nn) for (t0, nn) in blocks}
                pending_e3 = []
                proj_q = []
                # mtile m covers decode steps [8m, 8m+8); its h2 states are
                # written by the end of tt = NF + 8(m+1) - 1 + LAG
                proj_ready = {NF + 8 * (m + 1) + LAG: m for m in range(PROJ_IN_CHAIN)}
                for tt in range(NSTEP + LAG):
                    steps = []
                    if tt < NSTEP:
                        steps.append(h1_mm(tt))
                    s = tt - LAG
                    if 0 <= s < NSTEP:
                        steps.append(h2_mm(s))
                    gate_pair(steps)
                    if tt in block_end:
                        t0, nn = block_end[tt]
                        pending_e3.extend((t0, nn, m) for m in range(MC))
                    # drain rate: everything must be out LAG-nn tts after
                    # its block closed, and the queue must be empty at the
                    # loop end
                    npop = 3
                    if tt >= NSTEP:
                        npop = 4
                    for _ in range(min(npop, len(pending_e3))):
                        e3_piece(*pending_e3.pop(0))
                    # spread early projection mtiles into the chain tail
                    if tt in proj_ready:
                        mt = proj_ready[tt]
                        proj_q.append(("h2p", mt, 0))
                        proj_q.extend(("v", mt, v) for v in range(nvt))
                    # the latency-bound chain tail only absorbs ~1 piece/tt
                    # before the injected exp/cast ops stretch its serial path
                    nproj = 1
                    for _ in range(min(nproj, len(proj_q))):
                        kind, mt, v = proj_q.pop(0)
                        if kind == "h2p":
                            proj_h2p_piece(mt)
                        else:
                            proj_vtile_piece(mt, v)
                assert not pending_e3, f"{len(pending_e3)} e3 pieces left"
            # ---- rest of the projection + log_softmax ----
            # |logits| is bounded well below fp32 exp overflow here, so
            # log_softmax runs without the max shift: lp = x - ln(sum(e^x)).
            # Each mtile's softmax tail + output DMA overlaps the next
            # mtile's matmuls.
            with tc.tile_pool(name="pstage", bufs=3) as pstage:
                for kind, mt, v in proj_q:
                    if kind == "h2p":
                        proj_h2p_piece(mt)
                    else:
                        proj_vtile_piece(mt, v)
                proj_q.clear()
                for mt in range(len(mtiles)):
                    if mt not in logits_t:
                        proj_h2p_piece(mt)
                        for v in range(nvt):
                            proj_vtile_piece(mt, v)
                    proj_finish(mt, pstage)
    return nc



# ---------------------------------------------------------------------------
# Host side
# ---------------------------------------------------------------------------
def _bf16(a):
    return np.ascontiguousarray(a, dtype=np.float32).astype(ml_dtypes.bfloat16)


def _f32(a):
    return np.ascontiguousarray(a, dtype=np.float32)


def prep_inputs(cfg, vid_feats, target_variable, emb, w_ih1, w_hh1, b_ih1,
                b_hh1, w_ih2, w_hh2, b_ih2, b_hh2, out_w, out_b):
    """Build per-core input maps."""
    BS, MC, KH, DH = cfg["BS"], cfg["MC"], cfg["KH"], cfg["DH"]
    TD, NC = cfg["TD"], cfg["n_cores"]

    vid_feats = np.asarray(vid_feats, dtype=np.float32)
    target_variable = np.asarray(target_variable)
    emb = np.asarray(emb, dtype=np.float32)

    # replicated tensors
    if cfg["chain_mode"] == "fp8":
        def _chain_w(a):
            f8max = float(ml_dtypes.finfo(ml_dtypes.float8_e3m4).max)
            scaled = np.clip(np.asarray(a, dtype=np.float32) * WSCALE,
                             -f8max, f8max)
            return np.ascontiguousarray(scaled).astype(ml_dtypes.float8_e3m4)
    elif cfg["chain_mode"] == "fp32":
        _chain_w = _f32
    else:
        _chain_w = _bf16
    def _fp8e4(a, scale):
        m = float(ml_dtypes.finfo(ml_dtypes.float8_e4m3).max)
        scaled = np.clip(np.asarray(a, dtype=np.float32) * scale, -m, m)
        return np.ascontiguousarray(scaled).astype(ml_dtypes.float8_e4m3)

    def _pmajor(a, kv):
        """[kv*128, n] -> [128, kv*n] partition-major packing."""
        a = np.asarray(a)
        return a.reshape(kv, 128, a.shape[1]).transpose(1, 0, 2).reshape(
            128, -1)

    shared = {
        "w1T": _fp8e4(_pmajor(np.asarray(w_ih1, np.float32).T, cfg["KV"]),
                      64.0),
        "wh1T": _chain_w(np.asarray(w_hh1).T),
        "w2T": _bf16(np.asarray(w_ih2).T),
        "wh2T": _chain_w(np.asarray(w_hh2).T),
        "owf": _fp8e4(np.asarray(out_w).T, OWSCALE),
    }
    # combined biases: b_ih (+ b_hh for the r,z chunks; the n chunk of b_hh
    # is applied inside the gate, before the r multiply)
    def comb(bi, bh):
        c = np.asarray(bi, dtype=np.float32).copy()
        c[: 2 * DH] += np.asarray(bh, dtype=np.float32)[: 2 * DH]
        return c

    c1 = comb(b_ih1, b_hh1)
    c2 = comb(b_ih2, b_hh2)
    fast = cfg["chain_mode"] == "fp8" and not cfg["has_bhn"]
    # fast gate: gi (and thus its bias) for the r/z chunks is stored xHSCALE
    # for the identity-matmul accumulate into the psum
    shared["bi1c"] = _f32(c1.reshape(MC, 128).T)
    shared["bi2c"] = _f32(c2.reshape(MC, 128).T)
    gid = np.broadcast_to(c1.reshape(MC, 128).T[:, :, None],
                          (128, MC, BS))
    if fast:
        shared["gidec"] = _fp8e4(gid, 1.0)
    else:
        shared["gidec"] = _f32(gid)
    shared["bhn1"] = _f32(np.broadcast_to(
        np.asarray(b_hh1, np.float32)[2 * DH:].reshape(KH, 128).T[:, :, None],
        (128, KH, BS)))
    shared["bhn2"] = _f32(np.broadcast_to(
        np.asarray(b_hh2, np.float32)[2 * DH:].reshape(KH, 128).T[:, :, None],
        (128, KH, BS)))
    if cfg["has_out_b"]:
        # the fp8 projection psum carries a PSCALE*OWSCALE factor, so the
        # matmul-accumulated bias must carry it too
        shared["outb"] = _bf16(
            np.asarray(out_b).reshape(1, -1) * (PSCALE * OWSCALE))
        shared["ones"] = _bf16(np.ones((1, 128)))

    words = emb[np.asarray(target_variable[:, :TD], dtype=np.int64)]  # [B,TD,DW]
    if cfg["chain_mode"] == "fp8" and not cfg["has_bhn"]:
        # fp8 fast gate: h1 (E3's other rhs) is carried xHSCALE, so words
        # must match; the E3 bias ACT divides the psum back down
        words = words * HSCALE

    in_maps = []
    for c in range(NC):
        sl = slice(c * BS, (c + 1) * BS)
        vs = vid_feats[sl]                      # [BS, NF, DV]
        ws = words[sl]                          # [BS, TD, DW]
        m = dict(shared)
        m["xT"] = _fp8e4(
            _pmajor(vs.transpose(2, 1, 0).reshape(cfg["DV"], -1),
                    cfg["KV"]), 16.0)
        m["wordsT"] = _bf16(ws.transpose(2, 1, 0).reshape(cfg["DW"], -1))
        in_maps.append(m)
    return in_maps


_CACHE = {}
LAST_RESULT = None


CHAIN_MODE = "fp8"


def kernel(**inputs):
    global LAST_RESULT
    from concourse.bass_utils import run_bass_kernel_spmd

    out_b = np.asarray(inputs["out_b"])
    has_out_b = bool(np.any(out_b))
    DH = 512
    has_bhn = bool(np.any(np.asarray(inputs["b_hh1"])[2 * DH:])
                   or np.any(np.asarray(inputs["b_hh2"])[2 * DH:]))
    key = ("full", has_out_b, CHAIN_MODE, has_bhn)
    if key not in _CACHE:
        cfg = make_cfg(has_out_b=has_out_b, chain_mode=CHAIN_MODE,
                       has_bhn=has_bhn)
        _CACHE[key] = (cfg, build_nc(cfg))
    cfg, nc = _CACHE[key]

    in_maps = prep_inputs(cfg, **inputs)
    res = run_bass_kernel_spmd(nc, in_maps, list(range(cfg["n_cores"])))
    LAST_RESULT = res
    outs = [np.asarray(res.results[c]["out"]).astype(np.float32)
            for c in range(cfg["n_cores"])]
    return np.concatenate(outs, axis=0)  # [B, TD, V]

